# revision 20
# baseline (speedup 1.0000x reference)
"""Trainium2 Bass kernel for additive-attention nn.Module.

Math: reference computes
    scores[b,i,j] = x[b,i,:]@W[0,:3] + key[b,j,:]@W[0,3:] + b0
    attn = softmax(scores, axis=j) ; out = attn @ value

softmax over j is shift-invariant, so the x- and bias-terms (constant in j)
cancel exactly: attn[b,i,j] = softmax_j(key[b,j,:]@W[0,3:]) independent of i.
Hence out[b,i,:] = sum_j p[b,j] * value[b,j,:]  (identical for every i).

Kernel (data-parallel over batch, 8 batches/core on 8 cores). The per-core
work is a pure HBM stream: read 2 MB of fp8 value, weighted-reduce over j.

v5 structure. Measured facts driving it: a dma_start blocks its issuing
sequencer for a ~600-770 ns fixed DIRECT2D dispatch (descriptor count
barely matters), single-partition DVE ops cost ~0.5 us, and engine-to-
engine semaphore wakeups cost ~0.5-1 us.  So:
  - value moves as 8 whole-batch DMAs (256 KB each, 2 KB/partition
    descriptors): sync ring batches [0,1,2], scalar [3,4,5], gpsimd
    [kil,6,7].  Three rings drain concurrently at the HBM limit; batch
    completion order is ~[0,3,6,1,4,7,2,5] (chase order ARRIVAL).
  - per (batch, jj-chunk) the reduction is an M=1 matmul
      psum[1,256] += e_il[:, jj*8+b] (128x1 bf16) x v_chunk (128x256 fp8)
    at column group a%4 (a = arrival index), so 4 chase streams run
    concurrently on the PE; each arrival-quad accumulates into ONE psum
    tile at partitions {0,32,64,96}.
  - normalization: 1/s is routed to partition 32g once via
    PE-transpose(rr) -> mask -> block-indicator matmul, giving
    b8x[q,h] = 1/s[batch(g,h)]; each quad then normalizes with a single
    [4,256] partition-strided multiply and ships as one 4 KB DMA.
  - the e-chain exp issues on the scalar ring after its three value
    gens; a dummy Exp up front preloads the ACT table.
  - device output is out_d[4, 512] f32 (8 KB): row g col-half h = batch
    ARRIVAL[h*4+g].  The S1=1024 broadcast happens during host unshard.
"""

import numpy as np
import ml_dtypes
from contextlib import ExitStack

import concourse.bass as bass
import concourse.bacc as bacc
import concourse.mybir as mybir
from concourse import tile
from concourse.bass_utils import run_bass_kernel_spmd

B, S1, S2, DV = 64, 1024, 1024, 256
NCORES = 8
BPC = B // NCORES            # batches per core
NJ = S2 // 128               # j-chunks / row-interleave factor
F32 = mybir.dt.float32
BF16 = mybir.dt.bfloat16
FP8 = mybir.dt.float8e3
FP8_NP = ml_dtypes.float8_e3m4

SYNC_B = [0, 1, 2]
SCAL_B = [3, 4, 5]
GPS_B = [6, 7]
ARRIVAL = [0, 3, 6, 1, 4, 7, 2, 5]
N_WARM = 4

_compiled = {}


def _build_nc():
    nc = bacc.Bacc("TRN2", target_bir_lowering=False, debug=False,
                   num_devices=NCORES)

    kil_d = nc.dram_tensor("kil", [128, 195], F32, kind="ExternalInput")
    val_d = nc.dram_tensor("value", [BPC, S2, DV], FP8, kind="ExternalInput")
    out_d = nc.dram_tensor("out", [4, 2 * DV], F32, kind="ExternalOutput")

    with tile.TileContext(nc) as tc, ExitStack() as ctx:
        sm = ctx.enter_context(tc.tile_pool(name="sm", bufs=1))
        vpool = ctx.enter_context(tc.tile_pool(name="v", bufs=BPC))
        ps_warm = ctx.enter_context(
            tc.tile_pool(name="ps_warm", bufs=1, space=bass.MemorySpace.PSUM))
        ps_s = ctx.enter_context(
            tc.tile_pool(name="ps_s", bufs=1, space=bass.MemorySpace.PSUM))
        ps_b8 = ctx.enter_context(
            tc.tile_pool(name="ps_b8", bufs=1, space=bass.MemorySpace.PSUM))
        ps_v = ctx.enter_context(
            tc.tile_pool(name="ps_v", bufs=4, space=bass.MemorySpace.PSUM))

        kil_sb = sm.tile([128, 195], F32)
        dmy = sm.tile([1, 4], F32)
        dmy2 = sm.tile([1, 4], F32)
        warm = sm.tile([128, 256], BF16)
        ones_sb = sm.tile([128, BPC], BF16)
        ones8 = sm.tile([BPC, 128], F32)
        t0 = sm.tile([128, BPC * NJ], F32)
        t1 = sm.tile([128, BPC * NJ], F32)
        t2 = sm.tile([128, BPC * NJ], F32)
        e_il = sm.tile([128, BPC * NJ], BF16)
        s8 = sm.tile([BPC, BPC], F32)
        rr = sm.tile([BPC, BPC], F32)
        b8_sb = sm.tile([128, BPC], F32)
        o_sb = sm.tile([128, 2 * DV], F32)

        # ---- value stream: whole-batch DMAs, 3 rings ----
        nc.gpsimd.dma_start(kil_sb[:], kil_d[:])
        v_tiles = [None] * BPC
        for b in range(BPC):
            v_sb = vpool.tile([128, NJ * DV], FP8, tag="v_sb")
            v_tiles[b] = v_sb
        # dummy Exp first on scalar ring: ACT table preload (needs dmy)
        nc.vector.memset(dmy[:], 0.0)
        nc.scalar.activation(dmy2[:], dmy[:],
                             mybir.ActivationFunctionType.Exp,
                             bias=0.0, scale=1.0)
        for i in range(3):
            for blist, eng in ((SYNC_B, nc.sync), (SCAL_B, nc.scalar),
                               (GPS_B, nc.gpsimd)):
                if i < len(blist):
                    b = blist[i]
                    src = val_d.ap()[b].rearrange(
                        "(q jj) d -> q (jj d)", q=128)
                    eng.dma_start(v_tiles[b][:], src[:])

        nc.vector.memset(warm[:], 0.0)
        nc.vector.memset(ones_sb[:], 1.0)
        nc.vector.memset(ones8[:], 1.0 / BPC)

        # ---- PE warm-up (dependency-free, fills HAM activity window) ----
        wps = ps_warm.tile([BPC, 256], F32)
        for _ in range(N_WARM):
            nc.tensor.matmul(wps[:], warm[:, 0:BPC], warm[:],
                             start=True, stop=True)

        # ---- e_il[q, jj*8+b] = exp(key[b, 8q+jj, :] . w_k)  (bf16) ----
        wk_sb = kil_sb[:, 192:195]
        k3 = kil_sb[:, 0:192].rearrange("q (m f) -> q m f", f=3)
        nc.vector.tensor_scalar_mul(t0[:], k3[:, :, 0], wk_sb[:, 0:1])
        nc.vector.scalar_tensor_tensor(
            t1[:], k3[:, :, 1], wk_sb[:, 1:2], t0[:],
            op0=mybir.AluOpType.mult, op1=mybir.AluOpType.add)
        nc.vector.scalar_tensor_tensor(
            t2[:], k3[:, :, 2], wk_sb[:, 2:3], t1[:],
            op0=mybir.AluOpType.mult, op1=mybir.AluOpType.add)
        nc.scalar.activation(e_il[:], t2[:], mybir.ActivationFunctionType.Exp,
                             bias=0.0, scale=1.0)

        # ---- s[b] = sum_j e ; rr[p, b] = 1/s[b] on partitions 0..7 ----
        s_ps = ps_s.tile([BPC, BPC * NJ], F32)
        nc.tensor.matmul(s_ps[:], ones_sb[:], e_il[:], start=True, stop=True)
        nc.vector.tensor_reduce(
            s8[:], s_ps[:].rearrange("p (jj b) -> p b jj", b=BPC),
            axis=mybir.AxisListType.X, op=mybir.AluOpType.add)
        nc.vector.reciprocal(rr[:], s8[:])

        # ---- value reduction: chase batches, col group a%4 ----
        # one psum tile per arrival-pair (2 concurrent column-group
        # streams per PSUM bank; 4 corrupts)
        pair_ps = []
        for _p in range(4):
            ppt = ps_v.tile([128, DV], F32, tag="pair_ps")
            pair_ps.append(ppt)

        def vmm(a, jj):
            b = ARRIVAL[a]
            g = 32 * (a % 4)
            nc.tensor.matmul(
                pair_ps[a // 2][g:g + 1, :],
                e_il[:, jj * BPC + b:jj * BPC + b + 1],
                v_tiles[b][:, jj * DV:(jj + 1) * DV],
                start=(jj == 0), stop=(jj == NJ - 1),
                tile_position=(0, g))

        for jj in range(NJ):
            for a in range(4):
                vmm(a, jj)
        # broadcast 1/s down all partitions: B8[q, b] = 1/s[b]
        b8_ps = ps_b8.tile([128, BPC], F32)
        nc.tensor.matmul(b8_ps[:], ones8[:], rr[:], start=True, stop=True)
        nc.vector.tensor_copy(b8_sb[:], b8_ps[:])
        for jj in range(NJ):
            for a in range(4, BPC):
                vmm(a, jj)

        # ---- per-batch normalize spread over 3 engines + ship ----
        o_v = o_sb[:].rearrange("(g r) c -> g r c", g=4)
        norm_eng = [nc.vector, nc.scalar]
        for h in range(2):
            for i in range(4):
                a = h * 4 + i
                b = ARRIVAL[a]
                g = 32 * (a % 4)
                eng = norm_eng[a % 2]
                if eng is nc.scalar:
                    eng.mul(o_sb[g:g + 1, h * DV:(h + 1) * DV],
                            pair_ps[a // 2][g:g + 1, :],
                            b8_sb[g:g + 1, b:b + 1])
                else:
                    eng.tensor_scalar_mul(
                        o_sb[g:g + 1, h * DV:(h + 1) * DV],
                        pair_ps[a // 2][g:g + 1, :],
                        b8_sb[g:g + 1, b:b + 1])
            nc.sync.dma_start(out_d[:, h * DV:(h + 1) * DV],
                              o_v[:, 0, h * DV:(h + 1) * DV])

    nc.compile()
    return nc


def _get_nc():
    if "nc" not in _compiled:
        _compiled["nc"] = _build_nc()
    return _compiled["nc"]


def _make_in_maps(key, value, W):
    key = np.asarray(key, dtype=np.float32)
    value = np.asarray(value, dtype=np.float32)
    W = np.asarray(W, dtype=np.float32)
    vq = value.astype(FP8_NP)
    wk128 = np.tile(W[0, 3:].reshape(1, 3), (128, 1)).astype(np.float32)
    in_maps = []
    for c in range(NCORES):
        lo, hi = c * BPC, (c + 1) * BPC
        kc = key[lo:hi]                        # (BPC, S2, 3)
        # kil[q, (jj*BPC+b)*3+f] = key[b, interleaved row 8q+jj, f]
        kil = kc.reshape(BPC, 128, NJ, 3).transpose(1, 2, 0, 3)
        kil = kil.reshape(128, BPC * NJ * 3)
        kil = np.ascontiguousarray(np.concatenate([kil, wk128], axis=1))
        in_maps.append({
            "kil": kil,
            "value": np.ascontiguousarray(vq[lo:hi]),
        })
    return in_maps


def _finish(res):
    # device out[g, h*DV:...] = normalized row of batch ARRIVAL[h*4+g]
    parts = []
    for r in res.results:
        o = r["out"].reshape(4, 2 * DV)
        o8c = np.empty((BPC, DV), dtype=np.float32)
        for a in range(BPC):
            g, h = a % 4, a // 4
            o8c[ARRIVAL[a]] = o[g, h * DV:(h + 1) * DV]
        parts.append(o8c)
    o8 = np.concatenate(parts, axis=0)         # (B, DV)
    full = np.broadcast_to(o8[:, None, :], (B, S1, DV))
    return np.ascontiguousarray(full)


def kernel(x, key, value, W, b):
    nc = _get_nc()
    in_maps = _make_in_maps(key, value, W)
    res = run_bass_kernel_spmd(nc, in_maps, core_ids=list(range(NCORES)))
    return _finish(res)


def kernel_traced(x, key, value, W, b, **spmd_kwargs):
    """Like kernel() but returns (output, BassKernelResults) — for test.py."""
    nc = _get_nc()
    in_maps = _make_in_maps(key, value, W)
    res = run_bass_kernel_spmd(nc, in_maps, core_ids=list(range(NCORES)),
                               **spmd_kwargs)
    return _finish(res), res


# revision 21
# speedup vs baseline: 1.0042x; 1.0042x over previous
"""Trainium2 Bass kernel for additive-attention nn.Module.

Math: reference computes
    scores[b,i,j] = x[b,i,:]@W[0,:3] + key[b,j,:]@W[0,3:] + b0
    attn = softmax(scores, axis=j) ; out = attn @ value

softmax over j is shift-invariant, so the x- and bias-terms (constant in j)
cancel exactly: attn[b,i,j] = softmax_j(key[b,j,:]@W[0,3:]) independent of i.
Hence out[b,i,:] = sum_j p[b,j] * value[b,j,:]  (identical for every i).

Kernel (data-parallel over batch, 8 batches/core on 8 cores). The per-core
work is a pure HBM stream: read 2 MB of fp8 value, weighted-reduce over j.

v5 structure. Measured facts driving it: a dma_start blocks its issuing
sequencer for a ~600-770 ns fixed DIRECT2D dispatch (descriptor count
barely matters), single-partition DVE ops cost ~0.5 us, and engine-to-
engine semaphore wakeups cost ~0.5-1 us.  So:
  - value moves as 8 whole-batch DMAs (256 KB each, 2 KB/partition
    descriptors): sync ring batches [0,1,2], scalar [3,4,5], gpsimd
    [kil,6,7].  Three rings drain concurrently at the HBM limit; batch
    completion order is ~[0,3,6,1,4,7,2,5] (chase order ARRIVAL).
  - per (batch, jj-chunk) the reduction is an M=1 matmul
      psum[1,256] += e_il[:, jj*8+b] (128x1 bf16) x v_chunk (128x256 fp8)
    at column group a%4 (a = arrival index), so 4 chase streams run
    concurrently on the PE; each arrival-quad accumulates into ONE psum
    tile at partitions {0,32,64,96}.
  - normalization: 1/s is routed to partition 32g once via
    PE-transpose(rr) -> mask -> block-indicator matmul, giving
    b8x[q,h] = 1/s[batch(g,h)]; each quad then normalizes with a single
    [4,256] partition-strided multiply and ships as one 4 KB DMA.
  - the e-chain exp issues on the scalar ring after its three value
    gens; a dummy Exp up front preloads the ACT table.
  - device output is out_d[4, 512] f32 (8 KB): row g col-half h = batch
    ARRIVAL[h*4+g].  The S1=1024 broadcast happens during host unshard.
"""

import numpy as np
import ml_dtypes
from contextlib import ExitStack

import concourse.bass as bass
import concourse.bacc as bacc
import concourse.mybir as mybir
from concourse import tile
from concourse.bass_utils import run_bass_kernel_spmd

B, S1, S2, DV = 64, 1024, 1024, 256
NCORES = 8
BPC = B // NCORES            # batches per core
NJ = S2 // 128               # j-chunks / row-interleave factor
F32 = mybir.dt.float32
BF16 = mybir.dt.bfloat16
FP8 = mybir.dt.float8e3
FP8_NP = ml_dtypes.float8_e3m4

SYNC_B = [0, 1, 2]
SCAL_B = [3, 4, 5]
GPS_B = [6, 7]
ARRIVAL = [0, 3, 6, 1, 4, 7, 2, 5]
N_WARM = 4

_compiled = {}


def _build_nc():
    nc = bacc.Bacc("TRN2", target_bir_lowering=False, debug=False,
                   num_devices=NCORES)

    kil_d = nc.dram_tensor("kil", [128, 195], F32, kind="ExternalInput")
    val_d = nc.dram_tensor("value", [BPC, S2, DV], FP8, kind="ExternalInput")
    out_d = nc.dram_tensor("out", [4, 2 * DV], F32, kind="ExternalOutput")

    with tile.TileContext(nc) as tc, ExitStack() as ctx:
        sm = ctx.enter_context(tc.tile_pool(name="sm", bufs=1))
        vpool = ctx.enter_context(tc.tile_pool(name="v", bufs=BPC))
        ps_warm = ctx.enter_context(
            tc.tile_pool(name="ps_warm", bufs=1, space=bass.MemorySpace.PSUM))
        ps_s = ctx.enter_context(
            tc.tile_pool(name="ps_s", bufs=1, space=bass.MemorySpace.PSUM))
        ps_b8 = ctx.enter_context(
            tc.tile_pool(name="ps_b8", bufs=1, space=bass.MemorySpace.PSUM))
        ps_v = ctx.enter_context(
            tc.tile_pool(name="ps_v", bufs=4, space=bass.MemorySpace.PSUM))

        kil_sb = sm.tile([128, 195], F32)
        dmy = sm.tile([1, 4], F32)
        dmy2 = sm.tile([1, 4], F32)
        warm = sm.tile([128, 256], BF16)
        ones_sb = sm.tile([128, BPC], BF16)
        ones8 = sm.tile([BPC, 128], F32)
        t0 = sm.tile([128, BPC * NJ], F32)
        t1 = sm.tile([128, BPC * NJ], F32)
        t2 = sm.tile([128, BPC * NJ], F32)
        e_il = sm.tile([128, BPC * NJ], BF16)
        s8 = sm.tile([BPC, BPC], F32)
        rr = sm.tile([BPC, BPC], F32)
        b8_sb = sm.tile([128, BPC], F32)
        o_sb = sm.tile([128, 2 * DV], F32)

        # ---- value stream: whole-batch DMAs, 3 rings ----
        nc.gpsimd.dma_start(kil_sb[:], kil_d[:])
        v_tiles = [None] * BPC
        for b in range(BPC):
            v_sb = vpool.tile([128, NJ * DV], FP8, tag="v_sb")
            v_tiles[b] = v_sb
        # dummy Exp first on scalar ring: ACT table preload (needs dmy)
        nc.vector.memset(dmy[:], 0.0)
        nc.scalar.activation(dmy2[:], dmy[:],
                             mybir.ActivationFunctionType.Exp,
                             bias=0.0, scale=1.0)
        for i in range(3):
            for blist, eng in ((SYNC_B, nc.sync), (SCAL_B, nc.scalar),
                               (GPS_B, nc.gpsimd)):
                if i < len(blist):
                    b = blist[i]
                    src = val_d.ap()[b].rearrange(
                        "(q jj) d -> q (jj d)", q=128)
                    eng.dma_start(v_tiles[b][:], src[:])

        nc.vector.memset(warm[:], 0.0)
        nc.vector.memset(ones_sb[:], 1.0)
        nc.vector.memset(ones8[:], 1.0 / BPC)

        # ---- PE warm-up (dependency-free, fills HAM activity window) ----
        wps = ps_warm.tile([BPC, 256], F32)
        for _ in range(N_WARM):
            nc.tensor.matmul(wps[:], warm[:, 0:BPC], warm[:],
                             start=True, stop=True)

        # ---- e_il[q, jj*8+b] = exp(key[b, 8q+jj, :] . w_k)  (bf16) ----
        wk_sb = kil_sb[:, 192:195]
        k3 = kil_sb[:, 0:192].rearrange("q (m f) -> q m f", f=3)
        nc.vector.tensor_scalar_mul(t0[:], k3[:, :, 0], wk_sb[:, 0:1])
        nc.vector.scalar_tensor_tensor(
            t1[:], k3[:, :, 1], wk_sb[:, 1:2], t0[:],
            op0=mybir.AluOpType.mult, op1=mybir.AluOpType.add)
        nc.vector.scalar_tensor_tensor(
            t2[:], k3[:, :, 2], wk_sb[:, 2:3], t1[:],
            op0=mybir.AluOpType.mult, op1=mybir.AluOpType.add)
        nc.scalar.activation(e_il[:], t2[:], mybir.ActivationFunctionType.Exp,
                             bias=0.0, scale=1.0)

        # ---- s[b] = sum_j e ; rr[p, b] = 1/s[b] on partitions 0..7 ----
        s_ps = ps_s.tile([BPC, BPC * NJ], F32)
        nc.tensor.matmul(s_ps[:], ones_sb[:], e_il[:], start=True, stop=True)
        nc.vector.tensor_reduce(
            s8[:], s_ps[:].rearrange("p (jj b) -> p b jj", b=BPC),
            axis=mybir.AxisListType.X, op=mybir.AluOpType.add)
        nc.vector.reciprocal(rr[:], s8[:])

        # ---- value reduction: chase batches, col group a%4 ----
        # one psum tile per arrival-pair (2 concurrent column-group
        # streams per PSUM bank; 4 corrupts)
        pair_ps = []
        for _p in range(4):
            ppt = ps_v.tile([128, DV], F32, tag="pair_ps")
            pair_ps.append(ppt)

        def vmm(a, jj):
            b = ARRIVAL[a]
            g = 32 * (a % 4)
            nc.tensor.matmul(
                pair_ps[a // 2][g:g + 1, :],
                e_il[:, jj * BPC + b:jj * BPC + b + 1],
                v_tiles[b][:, jj * DV:(jj + 1) * DV],
                start=(jj == 0), stop=(jj == NJ - 1),
                tile_position=(0, g))

        # emit by arrival clump (one batch per ring lands ~together)
        for jj in range(NJ):
            for a in (0, 1, 2):
                vmm(a, jj)
        # broadcast 1/s down all partitions: B8[q, b] = 1/s[b]
        b8_ps = ps_b8.tile([128, BPC], F32)
        nc.tensor.matmul(b8_ps[:], ones8[:], rr[:], start=True, stop=True)
        nc.vector.tensor_copy(b8_sb[:], b8_ps[:])
        for jj in range(NJ):
            for a in (3, 4, 5):
                vmm(a, jj)
        for jj in range(NJ):
            for a in (6, 7):
                vmm(a, jj)

        # ---- per-batch normalize spread over 3 engines + ship ----
        o_v = o_sb[:].rearrange("(g r) c -> g r c", g=4)
        norm_eng = [nc.vector, nc.scalar]
        for h in range(2):
            for i in range(4):
                a = h * 4 + i
                b = ARRIVAL[a]
                g = 32 * (a % 4)
                eng = norm_eng[a % 2]
                if eng is nc.scalar:
                    eng.mul(o_sb[g:g + 1, h * DV:(h + 1) * DV],
                            pair_ps[a // 2][g:g + 1, :],
                            b8_sb[g:g + 1, b:b + 1])
                else:
                    eng.tensor_scalar_mul(
                        o_sb[g:g + 1, h * DV:(h + 1) * DV],
                        pair_ps[a // 2][g:g + 1, :],
                        b8_sb[g:g + 1, b:b + 1])
            nc.sync.dma_start(out_d[:, h * DV:(h + 1) * DV],
                              o_v[:, 0, h * DV:(h + 1) * DV])

    nc.compile()
    return nc


def _get_nc():
    if "nc" not in _compiled:
        _compiled["nc"] = _build_nc()
    return _compiled["nc"]


def _make_in_maps(key, value, W):
    key = np.asarray(key, dtype=np.float32)
    value = np.asarray(value, dtype=np.float32)
    W = np.asarray(W, dtype=np.float32)
    vq = value.astype(FP8_NP)
    wk128 = np.tile(W[0, 3:].reshape(1, 3), (128, 1)).astype(np.float32)
    in_maps = []
    for c in range(NCORES):
        lo, hi = c * BPC, (c + 1) * BPC
        kc = key[lo:hi]                        # (BPC, S2, 3)
        # kil[q, (jj*BPC+b)*3+f] = key[b, interleaved row 8q+jj, f]
        kil = kc.reshape(BPC, 128, NJ, 3).transpose(1, 2, 0, 3)
        kil = kil.reshape(128, BPC * NJ * 3)
        kil = np.ascontiguousarray(np.concatenate([kil, wk128], axis=1))
        in_maps.append({
            "kil": kil,
            "value": np.ascontiguousarray(vq[lo:hi]),
        })
    return in_maps


def _finish(res):
    # device out[g, h*DV:...] = normalized row of batch ARRIVAL[h*4+g]
    parts = []
    for r in res.results:
        o = r["out"].reshape(4, 2 * DV)
        o8c = np.empty((BPC, DV), dtype=np.float32)
        for a in range(BPC):
            g, h = a % 4, a // 4
            o8c[ARRIVAL[a]] = o[g, h * DV:(h + 1) * DV]
        parts.append(o8c)
    o8 = np.concatenate(parts, axis=0)         # (B, DV)
    full = np.broadcast_to(o8[:, None, :], (B, S1, DV))
    return np.ascontiguousarray(full)


def kernel(x, key, value, W, b):
    nc = _get_nc()
    in_maps = _make_in_maps(key, value, W)
    res = run_bass_kernel_spmd(nc, in_maps, core_ids=list(range(NCORES)))
    return _finish(res)


def kernel_traced(x, key, value, W, b, **spmd_kwargs):
    """Like kernel() but returns (output, BassKernelResults) — for test.py."""
    nc = _get_nc()
    in_maps = _make_in_maps(key, value, W)
    res = run_bass_kernel_spmd(nc, in_maps, core_ids=list(range(NCORES)),
                               **spmd_kwargs)
    return _finish(res), res


# revision 23
# speedup vs baseline: 1.0275x; 1.0232x over previous
"""Trainium2 Bass kernel for additive-attention nn.Module.

Math: reference computes
    scores[b,i,j] = x[b,i,:]@W[0,:3] + key[b,j,:]@W[0,3:] + b0
    attn = softmax(scores, axis=j) ; out = attn @ value

softmax over j is shift-invariant, so the x- and bias-terms (constant in j)
cancel exactly: attn[b,i,j] = softmax_j(key[b,j,:]@W[0,3:]) independent of i.
Hence out[b,i,:] = sum_j p[b,j] * value[b,j,:]  (identical for every i).

Kernel (data-parallel over batch, 8 batches/core on 8 cores). The per-core
work is a pure HBM stream: read 2 MB of fp8 value, weighted-reduce over j.

v6 structure.  Measured facts driving it: a dma_start blocks its issuing
sequencer ~600-770 ns (fixed DIRECT2D dispatch); cross-engine semaphore
wakeups cost ~0.5-1.5 us; a DVE op costs ~0.5 us regardless of partition
count (per-partition elements bound it); only {0,32,64,96} partition bases
and unit partition steps are legal for engine ops.
  - value: 8 whole-batch DMAs (256 KB, 2 KB/partition lines): sync ring
    [kil, v0, v1, v2, out1, out2], scalar [v3, v4, v5], gpsimd [v6, v7].
    Batches land in ring-clumps ~[0,3,6], [1,4,7], [2,5] (= ARRIVAL).
  - e-chain with minimal hops: kil first on sync (drains before value
    floods), dot products on the gpsimd engine (its ring is free after
    two Q7 dispatches), exp on scalar slotted after its value gens.
  - per (batch, jj-chunk): M=1 matmul psum[1,256] += e_il col x v chunk,
    column group a%4, one psum tile per arrival QUAD (rows 0/32/64/96),
    matmuls emitted jj-major per clump for column-group concurrency.
  - normalization: rr is transposed on the PE (identity from host),
    masked, and routed through a block-indicator matmul into
    b8x[q,h] = 1/s[batch(q//32, h)]; each half then normalizes+copies
    PSUM->SBUF in ONE [97,256] op (contiguous partitions, garbage rows
    scaled harmlessly) and ships as one 4 KB partition-strided DMA.
  - device output out_d[4, 512] f32: row g, col-half h = batch
    ARRIVAL[h*4+g].  The S1=1024 broadcast happens during host unshard.
"""

import numpy as np
import ml_dtypes
from contextlib import ExitStack

import concourse.bass as bass
import concourse.bacc as bacc
import concourse.mybir as mybir
from concourse import tile
from concourse.bass_utils import run_bass_kernel_spmd

B, S1, S2, DV = 64, 1024, 1024, 256
NCORES = 8
BPC = B // NCORES            # batches per core
NJ = S2 // 128               # j-chunks / row-interleave factor
F32 = mybir.dt.float32
BF16 = mybir.dt.bfloat16
FP8 = mybir.dt.float8e3
FP8_NP = ml_dtypes.float8_e3m4

SYNC_B = [0, 1, 2]
SCAL_B = [3, 4, 5]
GPS_B = [6, 7]
ARRIVAL = [0, 3, 6, 1, 4, 7, 2, 5]
CLUMPS = [(0, 1, 2), (3, 4, 5), (6, 7)]
N_WARM = 4
KW = 333                     # kil columns: 192 key + 3 w_k + 138 consts

_compiled = {}


def _build_nc():
    nc = bacc.Bacc("TRN2", target_bir_lowering=False, debug=False,
                   num_devices=NCORES)

    kil_d = nc.dram_tensor("kil", [128, KW], F32, kind="ExternalInput")
    val_d = nc.dram_tensor("value", [BPC, S2, DV], FP8, kind="ExternalInput")
    out_d = nc.dram_tensor("out", [4, 2 * DV], F32, kind="ExternalOutput")

    with tile.TileContext(nc) as tc, ExitStack() as ctx:
        sm = ctx.enter_context(tc.tile_pool(name="sm", bufs=1))
        vpool = ctx.enter_context(tc.tile_pool(name="v", bufs=BPC))
        ps_warm = ctx.enter_context(
            tc.tile_pool(name="ps_warm", bufs=1, space=bass.MemorySpace.PSUM))
        ps_s = ctx.enter_context(
            tc.tile_pool(name="ps_s", bufs=1, space=bass.MemorySpace.PSUM))
        ps_t = ctx.enter_context(
            tc.tile_pool(name="ps_t", bufs=1, space=bass.MemorySpace.PSUM))
        ps_b8 = ctx.enter_context(
            tc.tile_pool(name="ps_b8", bufs=1, space=bass.MemorySpace.PSUM))
        ps_v = ctx.enter_context(
            tc.tile_pool(name="ps_v", bufs=2, space=bass.MemorySpace.PSUM))

        kil_sb = sm.tile([128, KW], F32)
        dmy = sm.tile([1, 4], F32)
        dmy2 = sm.tile([1, 4], F32)
        warm = sm.tile([128, 256], BF16)
        ones_sb = sm.tile([128, BPC], BF16)
        t0 = sm.tile([128, BPC * NJ], F32)
        t1 = sm.tile([128, BPC * NJ], F32)
        t2 = sm.tile([128, BPC * NJ], F32)
        e_il = sm.tile([128, BPC * NJ], BF16)
        s8 = sm.tile([BPC, BPC], F32)
        rr = sm.tile([BPC, BPC], F32)
        rrx = sm.tile([BPC, 2], F32)
        b8x = sm.tile([128, 2], F32)
        o_sb = sm.tile([128, 2 * DV], F32)

        # ---- DMAs: kil first on sync, then value on all three rings ----
        nc.sync.dma_start(kil_sb[:], kil_d[:])
        v_tiles = [None] * BPC
        for b in range(BPC):
            v_sb = vpool.tile([128, NJ * DV], FP8, tag="v_sb")
            v_tiles[b] = v_sb
        nc.vector.memset(dmy[:], 0.0)
        nc.scalar.activation(dmy2[:], dmy[:],
                             mybir.ActivationFunctionType.Exp,
                             bias=0.0, scale=1.0)
        for i in range(3):
            for blist, eng in ((SYNC_B, nc.sync), (SCAL_B, nc.scalar),
                               (GPS_B, nc.gpsimd)):
                if i < len(blist):
                    b = blist[i]
                    src = val_d.ap()[b].rearrange(
                        "(q jj) d -> q (jj d)", q=128)
                    eng.dma_start(v_tiles[b][:], src[:])

        nc.vector.memset(warm[:], 0.0)
        nc.vector.memset(ones_sb[:], 1.0)
        wk_sb = kil_sb[:, 192:195]
        id8 = kil_sb[0:BPC, 195:203]
        blk = kil_sb[0:BPC, 203:331]
        maskh = kil_sb[0:BPC, 331:333]

        # ---- PE warm-up (dependency-free, fills HAM activity window) ----
        wps = ps_warm.tile([BPC, 256], F32)
        for _ in range(N_WARM):
            nc.tensor.matmul(wps[:], warm[:, 0:BPC], warm[:],
                             start=True, stop=True)

        # ---- e_il[q, jj*8+b] = exp(key[b, 8q+jj, :] . w_k)  (bf16) ----
        # dots on the gpsimd engine: its ring is free right after the two
        # value dispatches, so no wake contention with the vector chain
        k3 = kil_sb[:, 0:192].rearrange("q (m f) -> q m f", f=3)
        nc.vector.tensor_scalar_mul(t0[:], k3[:, :, 0], wk_sb[:, 0:1])
        nc.vector.scalar_tensor_tensor(
            t1[:], k3[:, :, 1], wk_sb[:, 1:2], t0[:],
            op0=mybir.AluOpType.mult, op1=mybir.AluOpType.add)
        nc.vector.scalar_tensor_tensor(
            t2[:], k3[:, :, 2], wk_sb[:, 2:3], t1[:],
            op0=mybir.AluOpType.mult, op1=mybir.AluOpType.add)
        nc.scalar.activation(e_il[:], t2[:], mybir.ActivationFunctionType.Exp,
                             bias=0.0, scale=1.0)

        # ---- value reduction + normalization-broadcast chain ----
        quad_ps = []
        for _q in range(2):
            qpt = ps_v.tile([128, DV], F32, tag="quad_ps")
            quad_ps.append(qpt)

        def vmm(a, jj):
            b = ARRIVAL[a]
            g = 32 * (a % 4)
            nc.tensor.matmul(
                quad_ps[a // 4][g:g + 1, :],
                e_il[:, jj * BPC + b:jj * BPC + b + 1],
                v_tiles[b][:, jj * DV:(jj + 1) * DV],
                start=(jj == 0), stop=(jj == NJ - 1),
                tile_position=(0, g))

        # s[b] = sum_j e (ones matmul; first in FIFO, needs only e_il)
        s_ps = ps_s.tile([BPC, BPC * NJ], F32)
        nc.tensor.matmul(s_ps[:], ones_sb[:], e_il[:], start=True, stop=True)
        nc.vector.tensor_reduce(
            s8[:], s_ps[:].rearrange("p (jj b) -> p b jj", b=BPC),
            axis=mybir.AxisListType.X, op=mybir.AluOpType.add)
        nc.vector.reciprocal(rr[:], s8[:])

        for jj in range(NJ):
            for a in CLUMPS[0]:
                vmm(a, jj)
        # rr^T on the PE: rrt[b, c] = 1/s[b] on partition b
        rrt_ps = ps_t.tile([BPC, BPC], F32)
        nc.tensor.transpose(rrt_ps[:], rr[:], id8)
        nc.vector.tensor_mul(rrx[:], rrt_ps[:, 0:2], maskh)
        for jj in range(NJ):
            for a in CLUMPS[1]:
                vmm(a, jj)
        # b8x[q, h] = 1/s[batch at (q//32, h)]
        b8_ps = ps_b8.tile([128, 2], F32)
        nc.tensor.matmul(b8_ps[:], blk, rrx[:], start=True, stop=True)
        nc.vector.tensor_copy(b8x[:], b8_ps[:])
        for jj in range(NJ):
            for a in CLUMPS[2]:
                vmm(a, jj)

        # ---- normalize+copy each half in one [97,256] op + ship ----
        o_v = o_sb[:].rearrange("(g r) c -> g r c", g=4)
        nc.vector.tensor_scalar_mul(
            o_sb[0:97, 0:DV], quad_ps[0][0:97, :], b8x[0:97, 0:1])
        nc.sync.dma_start(out_d[:, 0:DV], o_v[:, 0, 0:DV])
        nc.scalar.mul(
            o_sb[0:97, DV:2 * DV], quad_ps[1][0:97, :], b8x[0:97, 1:2])
        nc.sync.dma_start(out_d[:, DV:2 * DV], o_v[:, 0, DV:2 * DV])

    nc.compile()
    return nc


def _get_nc():
    if "nc" not in _compiled:
        _compiled["nc"] = _build_nc()
    return _compiled["nc"]


def _make_in_maps(key, value, W):
    key = np.asarray(key, dtype=np.float32)
    value = np.asarray(value, dtype=np.float32)
    W = np.asarray(W, dtype=np.float32)
    vq = value.astype(FP8_NP)
    wk128 = np.tile(W[0, 3:].reshape(1, 3), (128, 1)).astype(np.float32)
    # constant tiles (meaningful on partitions 0..7 only):
    # id8 (8), blk (128), maskh (2)
    consts = np.zeros((128, 138), dtype=np.float32)
    consts[0:BPC, 0:BPC] = np.eye(BPC, dtype=np.float32)
    for k in range(BPC):
        a = ARRIVAL.index(k)
        g, h = a % 4, a // 4
        consts[k, BPC + 32 * g:BPC + 32 * g + 32] = 1.0
        consts[k, 136 + h] = 1.0
    in_maps = []
    for c in range(NCORES):
        lo, hi = c * BPC, (c + 1) * BPC
        kc = key[lo:hi]                        # (BPC, S2, 3)
        # kil[q, (jj*BPC+b)*3+f] = key[b, interleaved row 8q+jj, f]
        kil = kc.reshape(BPC, 128, NJ, 3).transpose(1, 2, 0, 3)
        kil = kil.reshape(128, BPC * NJ * 3)
        kil = np.ascontiguousarray(
            np.concatenate([kil, wk128, consts], axis=1))
        in_maps.append({
            "kil": kil,
            "value": np.ascontiguousarray(vq[lo:hi]),
        })
    return in_maps


def _finish(res):
    # device out[g, h*DV:...] = normalized row of batch ARRIVAL[h*4+g]
    parts = []
    for r in res.results:
        o = r["out"].reshape(4, 2 * DV)
        o8c = np.empty((BPC, DV), dtype=np.float32)
        for a in range(BPC):
            g, h = a % 4, a // 4
            o8c[ARRIVAL[a]] = o[g, h * DV:(h + 1) * DV]
        parts.append(o8c)
    o8 = np.concatenate(parts, axis=0)         # (B, DV)
    full = np.broadcast_to(o8[:, None, :], (B, S1, DV))
    return np.ascontiguousarray(full)


def kernel(x, key, value, W, b):
    nc = _get_nc()
    in_maps = _make_in_maps(key, value, W)
    res = run_bass_kernel_spmd(nc, in_maps, core_ids=list(range(NCORES)))
    return _finish(res)


def kernel_traced(x, key, value, W, b, **spmd_kwargs):
    """Like kernel() but returns (output, BassKernelResults) — for test.py."""
    nc = _get_nc()
    in_maps = _make_in_maps(key, value, W)
    res = run_bass_kernel_spmd(nc, in_maps, core_ids=list(range(NCORES)),
                               **spmd_kwargs)
    return _finish(res), res


# revision 24
# speedup vs baseline: 1.1027x; 1.0732x over previous
"""Trainium2 Bass kernel for additive-attention nn.Module.

Math: reference computes
    scores[b,i,j] = x[b,i,:]@W[0,:3] + key[b,j,:]@W[0,3:] + b0
    attn = softmax(scores, axis=j) ; out = attn @ value

softmax over j is shift-invariant, so the x- and bias-terms (constant in j)
cancel exactly: attn[b,i,j] = softmax_j(key[b,j,:]@W[0,3:]) independent of i.
Hence out[b,i,:] = sum_j p[b,j] * value[b,j,:]  (identical for every i).

Kernel (data-parallel over batch, 8 batches/core on 8 cores). The per-core
work is a pure HBM stream: read 2 MB of fp8 value, weighted-reduce over j.

v6 structure.  Measured facts driving it: a dma_start blocks its issuing
sequencer ~600-770 ns (fixed DIRECT2D dispatch); cross-engine semaphore
wakeups cost ~0.5-1.5 us; a DVE op costs ~0.5 us regardless of partition
count (per-partition elements bound it); only {0,32,64,96} partition bases
and unit partition steps are legal for engine ops.
  - value: 8 whole-batch DMAs (256 KB, 2 KB/partition lines): sync ring
    [kil, v0, v1, v2, out1, out2], scalar [v3, v4, v5], gpsimd [v6, v7].
    Batches land in ring-clumps ~[0,3,6], [1,4,7], [2,5] (= ARRIVAL).
  - e-chain with minimal hops: kil first on sync (drains before value
    floods), dot products on the gpsimd engine (its ring is free after
    two Q7 dispatches), exp on scalar slotted after its value gens.
  - per (batch, jj-chunk): M=1 matmul psum[1,256] += e_il col x v chunk,
    column group a%4, one psum tile per arrival QUAD (rows 0/32/64/96),
    matmuls emitted jj-major per clump for column-group concurrency.
  - normalization: rr is transposed on the PE (identity from host),
    masked, and routed through a block-indicator matmul into
    b8x[q,h] = 1/s[batch(q//32, h)]; each half then normalizes+copies
    PSUM->SBUF in ONE [97,256] op (contiguous partitions, garbage rows
    scaled harmlessly) and ships as one 4 KB partition-strided DMA.
  - device output out_d[4, 512] f32: row g, col-half h = batch
    ARRIVAL[h*4+g].  The S1=1024 broadcast happens during host unshard.
"""

import numpy as np
import ml_dtypes
from contextlib import ExitStack

import concourse.bass as bass
import concourse.bacc as bacc
import concourse.mybir as mybir
from concourse import tile
from concourse.bass_utils import run_bass_kernel_spmd

B, S1, S2, DV = 64, 1024, 1024, 256
NCORES = 8
BPC = B // NCORES            # batches per core
NJ = S2 // 128               # j-chunks / row-interleave factor
F32 = mybir.dt.float32
BF16 = mybir.dt.bfloat16
FP8 = mybir.dt.float8e3
FP8_NP = ml_dtypes.float8_e3m4

SYNC_B = [0, 1, 2, 3]
SCAL_B = [4, 5, 6, 7]
GPS_B = []
ARRIVAL = [0, 4, 1, 5, 2, 6, 3, 7]
CLUMPS = [(0, 1), (2, 3), (4, 5), (6, 7)]
N_WARM = 4
KW = 333                     # kil columns: 192 key + 3 w_k + 138 consts

_compiled = {}


def _build_nc():
    nc = bacc.Bacc("TRN2", target_bir_lowering=False, debug=False,
                   num_devices=NCORES)

    kil_d = nc.dram_tensor("kil", [128, KW], F32, kind="ExternalInput")
    val_d = nc.dram_tensor("value", [BPC, S2, DV], FP8, kind="ExternalInput")
    out_d = nc.dram_tensor("out", [4, 2 * DV], F32, kind="ExternalOutput")

    with tile.TileContext(nc) as tc, ExitStack() as ctx:
        sm = ctx.enter_context(tc.tile_pool(name="sm", bufs=1))
        vpool = ctx.enter_context(tc.tile_pool(name="v", bufs=BPC))
        ps_warm = ctx.enter_context(
            tc.tile_pool(name="ps_warm", bufs=1, space=bass.MemorySpace.PSUM))
        ps_s = ctx.enter_context(
            tc.tile_pool(name="ps_s", bufs=1, space=bass.MemorySpace.PSUM))
        ps_t = ctx.enter_context(
            tc.tile_pool(name="ps_t", bufs=1, space=bass.MemorySpace.PSUM))
        ps_b8 = ctx.enter_context(
            tc.tile_pool(name="ps_b8", bufs=1, space=bass.MemorySpace.PSUM))
        ps_v = ctx.enter_context(
            tc.tile_pool(name="ps_v", bufs=2, space=bass.MemorySpace.PSUM))

        kil_sb = sm.tile([128, KW], F32)
        dmy = sm.tile([1, 4], F32)
        dmy2 = sm.tile([1, 4], F32)
        warm = sm.tile([128, 256], BF16)
        ones_sb = sm.tile([128, BPC], BF16)
        t0 = sm.tile([128, BPC * NJ], F32)
        t1 = sm.tile([128, BPC * NJ], F32)
        t2 = sm.tile([128, BPC * NJ], F32)
        e_il = sm.tile([128, BPC * NJ], BF16)
        s8 = sm.tile([BPC, BPC], F32)
        rr = sm.tile([BPC, BPC], F32)
        rrx = sm.tile([BPC, 2], F32)
        b8x = sm.tile([128, 2], F32)
        o_sb = sm.tile([128, 2 * DV], F32)

        # ---- DMAs: kil first on sync, then value on all three rings ----
        nc.sync.dma_start(kil_sb[:], kil_d[:])
        v_tiles = [None] * BPC
        for b in range(BPC):
            v_sb = vpool.tile([128, NJ * DV], FP8, tag="v_sb")
            v_tiles[b] = v_sb
        nc.vector.memset(dmy[:], 0.0)
        nc.scalar.activation(dmy2[:], dmy[:],
                             mybir.ActivationFunctionType.Exp,
                             bias=0.0, scale=1.0)
        for i in range(4):
            for blist, eng in ((SYNC_B, nc.sync), (SCAL_B, nc.scalar)):
                if i < len(blist):
                    b = blist[i]
                    src = val_d.ap()[b].rearrange(
                        "(q jj) d -> q (jj d)", q=128)
                    eng.dma_start(v_tiles[b][:], src[:])

        nc.vector.memset(warm[:], 0.0)
        nc.vector.memset(ones_sb[:], 1.0)
        wk_sb = kil_sb[:, 192:195]
        id8 = kil_sb[0:BPC, 195:203]
        blk = kil_sb[0:BPC, 203:331]
        maskh = kil_sb[0:BPC, 331:333]

        # ---- PE warm-up (dependency-free, fills HAM activity window) ----
        wps = ps_warm.tile([BPC, 256], F32)
        for _ in range(N_WARM):
            nc.tensor.matmul(wps[:], warm[:, 0:BPC], warm[:],
                             start=True, stop=True)

        # ---- e_il[q, jj*8+b] = exp(key[b, 8q+jj, :] . w_k)  (bf16) ----
        # dots on the gpsimd engine: its ring is free right after the two
        # value dispatches, so no wake contention with the vector chain
        k3 = kil_sb[:, 0:192].rearrange("q (m f) -> q m f", f=3)
        nc.vector.tensor_scalar_mul(t0[:], k3[:, :, 0], wk_sb[:, 0:1])
        nc.vector.scalar_tensor_tensor(
            t1[:], k3[:, :, 1], wk_sb[:, 1:2], t0[:],
            op0=mybir.AluOpType.mult, op1=mybir.AluOpType.add)
        nc.vector.scalar_tensor_tensor(
            t2[:], k3[:, :, 2], wk_sb[:, 2:3], t1[:],
            op0=mybir.AluOpType.mult, op1=mybir.AluOpType.add)
        nc.scalar.activation(e_il[:], t2[:], mybir.ActivationFunctionType.Exp,
                             bias=0.0, scale=1.0)

        # ---- value reduction + normalization-broadcast chain ----
        quad_ps = []
        for _q in range(2):
            qpt = ps_v.tile([128, DV], F32, tag="quad_ps")
            quad_ps.append(qpt)

        def vmm(a, jj):
            b = ARRIVAL[a]
            g = 32 * (a % 4)
            nc.tensor.matmul(
                quad_ps[a // 4][g:g + 1, :],
                e_il[:, jj * BPC + b:jj * BPC + b + 1],
                v_tiles[b][:, jj * DV:(jj + 1) * DV],
                start=(jj == 0), stop=(jj == NJ - 1),
                tile_position=(0, g))

        # s[b] = sum_j e (ones matmul; first in FIFO, needs only e_il)
        s_ps = ps_s.tile([BPC, BPC * NJ], F32)
        nc.tensor.matmul(s_ps[:], ones_sb[:], e_il[:], start=True, stop=True)
        nc.vector.tensor_reduce(
            s8[:], s_ps[:].rearrange("p (jj b) -> p b jj", b=BPC),
            axis=mybir.AxisListType.X, op=mybir.AluOpType.add)
        nc.vector.reciprocal(rr[:], s8[:])

        for jj in range(NJ):
            for a in CLUMPS[0]:
                vmm(a, jj)
        # rr^T on the PE: rrt[b, c] = 1/s[b] on partition b
        rrt_ps = ps_t.tile([BPC, BPC], F32)
        nc.tensor.transpose(rrt_ps[:], rr[:], id8)
        nc.vector.tensor_mul(rrx[:], rrt_ps[:, 0:2], maskh)
        for jj in range(NJ):
            for a in CLUMPS[1]:
                vmm(a, jj)
        # b8x[q, h] = 1/s[batch at (q//32, h)]
        b8_ps = ps_b8.tile([128, 2], F32)
        nc.tensor.matmul(b8_ps[:], blk, rrx[:], start=True, stop=True)
        nc.vector.tensor_copy(b8x[:], b8_ps[:])
        for jj in range(NJ):
            for a in CLUMPS[2]:
                vmm(a, jj)
        for jj in range(NJ):
            for a in CLUMPS[3]:
                vmm(a, jj)

        # ---- normalize+copy each half in one [97,256] op + ship ----
        o_v = o_sb[:].rearrange("(g r) c -> g r c", g=4)
        nc.vector.tensor_scalar_mul(
            o_sb[0:97, 0:DV], quad_ps[0][0:97, :], b8x[0:97, 0:1])
        nc.sync.dma_start(out_d[:, 0:DV], o_v[:, 0, 0:DV])
        nc.scalar.mul(
            o_sb[0:97, DV:2 * DV], quad_ps[1][0:97, :], b8x[0:97, 1:2])
        nc.sync.dma_start(out_d[:, DV:2 * DV], o_v[:, 0, DV:2 * DV])

    nc.compile()
    return nc


def _get_nc():
    if "nc" not in _compiled:
        _compiled["nc"] = _build_nc()
    return _compiled["nc"]


def _make_in_maps(key, value, W):
    key = np.asarray(key, dtype=np.float32)
    value = np.asarray(value, dtype=np.float32)
    W = np.asarray(W, dtype=np.float32)
    vq = value.astype(FP8_NP)
    wk128 = np.tile(W[0, 3:].reshape(1, 3), (128, 1)).astype(np.float32)
    # constant tiles (meaningful on partitions 0..7 only):
    # id8 (8), blk (128), maskh (2)
    consts = np.zeros((128, 138), dtype=np.float32)
    consts[0:BPC, 0:BPC] = np.eye(BPC, dtype=np.float32)
    for k in range(BPC):
        a = ARRIVAL.index(k)
        g, h = a % 4, a // 4
        consts[k, BPC + 32 * g:BPC + 32 * g + 32] = 1.0
        consts[k, 136 + h] = 1.0
    in_maps = []
    for c in range(NCORES):
        lo, hi = c * BPC, (c + 1) * BPC
        kc = key[lo:hi]                        # (BPC, S2, 3)
        # kil[q, (jj*BPC+b)*3+f] = key[b, interleaved row 8q+jj, f]
        kil = kc.reshape(BPC, 128, NJ, 3).transpose(1, 2, 0, 3)
        kil = kil.reshape(128, BPC * NJ * 3)
        kil = np.ascontiguousarray(
            np.concatenate([kil, wk128, consts], axis=1))
        in_maps.append({
            "kil": kil,
            "value": np.ascontiguousarray(vq[lo:hi]),
        })
    return in_maps


def _finish(res):
    # device out[g, h*DV:...] = normalized row of batch ARRIVAL[h*4+g]
    parts = []
    for r in res.results:
        o = r["out"].reshape(4, 2 * DV)
        o8c = np.empty((BPC, DV), dtype=np.float32)
        for a in range(BPC):
            g, h = a % 4, a // 4
            o8c[ARRIVAL[a]] = o[g, h * DV:(h + 1) * DV]
        parts.append(o8c)
    o8 = np.concatenate(parts, axis=0)         # (B, DV)
    full = np.broadcast_to(o8[:, None, :], (B, S1, DV))
    return np.ascontiguousarray(full)


def kernel(x, key, value, W, b):
    nc = _get_nc()
    in_maps = _make_in_maps(key, value, W)
    res = run_bass_kernel_spmd(nc, in_maps, core_ids=list(range(NCORES)))
    return _finish(res)


def kernel_traced(x, key, value, W, b, **spmd_kwargs):
    """Like kernel() but returns (output, BassKernelResults) — for test.py."""
    nc = _get_nc()
    in_maps = _make_in_maps(key, value, W)
    res = run_bass_kernel_spmd(nc, in_maps, core_ids=list(range(NCORES)),
                               **spmd_kwargs)
    return _finish(res), res
